# revision 44
# baseline (speedup 1.0000x reference)
"""CfC (Closed-form Continuous-time) RNN kernel for Trainium2 (Bass/Tile).

Reference semantics (per timestep t, batch B, input I=64, units U=128, backbone BB=256):
    z   = lecun_tanh(concat(x_t, h) @ W_bb + b_bb)        # lecun_tanh(v) = 1.7159*tanh(0.666*v)
    ff1 = tanh(z @ W_ff1 + b_ff1)
    ff2 = tanh(z @ W_ff2 + b_ff2)
    t_i = sigmoid(z @ W_ta + b_ta + z @ W_tb + b_tb)      # ts == 1.0
    h   = ff1 * (1 - t_i) + t_i * ff2
    y_t = h

Sharding: data-parallel, batch 256 -> 32 per core across 8 cores; weights replicated.

On-chip layout is feature-major ([feature, batch] tiles).  Host-side folds:
  - x is pre-transposed to [I+1, T*32] per core with a trailing ones row, so the
    backbone bias rides in the x-projection matmul (W1x has b_bb as its last row).
  - lecun_tanh's 1.7159 output scale is folded into the head weights.
  - sigmoid(a) = 0.5 + 0.5*tanh(a/2): the two t-heads are combined into one
    tanh head with weights 0.5*(W_ta+W_tb) (so all three heads share one tanh
    activation instruction), and the 0.5's are folded into the blend:
        2h = 2*ff1 + (1+tau)*(ff2-ff1),   tau = tanh(a/2)
  - the recurrent state is stored as hh = 2h; W1h is pre-scaled by 0.5 and the
    final y copy scales by 0.5.

The x-projection (x_t @ W1x + b_bb) for 16 steps at a time is computed by two
N=512 matmuls directly into a PSUM double-bank; the per-step recurrent matmuls
accumulate onto it (start=False), so no adds/copies are spent on it.
"""

import os

import numpy as np
import ml_dtypes

import concourse.bass as bass
import concourse.mybir as mybir
from concourse.bass import ts
from concourse.tile import TileContext, add_dep_helper
from concourse.masks import make_identity

B, T, I, U, BB = 256, 1024, 64, 128, 256
NCORES = 8
BC = B // NCORES          # 32 batch per core
NH = 384                  # fused head width: ff1 | ff2 | tcomb
GS = 16                   # steps per x-projection group (512 psum cols / 32 batch)
YB = 4                    # steps per transpose block (4*32 = 128 partitions)

F32 = mybir.dt.float32
BF16 = mybir.dt.bfloat16

_CACHE = {}

_PSEUDO = ("EventSemaphore", "Branch", "TileRelease", "ISA", "Nop")
_SCAN_PSEUDO = ("Drain",) + _PSEUDO


def _minimize_waits(nc):
    """Drop transitively-redundant semaphore waits.

    Tile's wait assignment is per-proc minimal but not transitively minimal
    across procs, and this walrus build rejects executable instructions with
    more than one sync wait.  Each semaphore here is updated by a single
    queue, and the scheduled instruction order is an admissible
    linearization, so we can propagate vector clocks in listed order and
    delete any wait already implied by (a) the issuing queue's own history
    or (b) another wait on the same instruction.
    """
    import bass_rust

    sem_hist = {}     # sem name -> list of (cum_value, clock dict)
    queue_clock = {}  # queue key -> clock dict (sem -> value)

    def clock_at(sem, val):
        h = sem_hist.get(sem)
        if not h:
            return None
        for cum, clk in h:
            if cum >= val:
                return clk
        return None

    def join(dst, src):
        for s, v in src.items():
            if dst.get(s, -1) < v:
                dst[s] = v

    # Pass A: build the complete semaphore history (clocks reflect the full
    # happens-before relation; drops in pass B only remove redundant edges,
    # so these clocks stay valid).
    for blk in nc.m.functions[0].blocks:
        for inst in blk.instructions:
            si = inst.sync_info
            if si is None:
                continue
            q = str(inst.engine)
            clk = dict(queue_clock.get(q, {}))
            for w in si.on_wait:
                if w.wait_mode == "sem-ge-imm" and w.wait_reg is None:
                    wc = clock_at(w.ant_name, w.wait_value)
                    if wc:
                        join(clk, wc)
                    if clk.get(w.ant_name, -1) < w.wait_value:
                        clk[w.ant_name] = w.wait_value
            for u in si.on_update:
                if u.update_mode == "sem-inc" and u.update_reg is None:
                    h = sem_hist.setdefault(u.ant_name, [])
                    cum = (h[-1][0] if h else 0) + u.update_value
                    uclk = dict(clk)
                    uclk[u.ant_name] = cum
                    h.append((cum, uclk))
            queue_clock[q] = clk
    # NOTE: waits on later-listed producers leave pass-A clocks incomplete
    # for those entries; run pass A once more so clock_at resolves forward
    # references (two iterations suffice for the patterns here).
    sem_hist2 = {}
    queue_clock = {}
    for blk in nc.m.functions[0].blocks:
        for inst in blk.instructions:
            si = inst.sync_info
            if si is None:
                continue
            q = str(inst.engine)
            clk = dict(queue_clock.get(q, {}))
            for w in si.on_wait:
                if w.wait_mode == "sem-ge-imm" and w.wait_reg is None:
                    wc = clock_at(w.ant_name, w.wait_value)
                    if wc:
                        join(clk, wc)
                    if clk.get(w.ant_name, -1) < w.wait_value:
                        clk[w.ant_name] = w.wait_value
            for u in si.on_update:
                if u.update_mode == "sem-inc" and u.update_reg is None:
                    h = sem_hist2.setdefault(u.ant_name, [])
                    cum = (h[-1][0] if h else 0) + u.update_value
                    uclk = dict(clk)
                    uclk[u.ant_name] = cum
                    h.append((cum, uclk))
            queue_clock[q] = clk
    sem_hist = sem_hist2

    queue_clock = {}
    n_multi = 0
    for blk in nc.m.functions[0].blocks:
        block_insts = list(blk.instructions)
        # same-queue successor memsets from the same source line, for
        # spreading excess DMA waits (see the y_sb padding-column absorbers)
        succ_memsets = {}
        for idx, inst in enumerate(block_insts):
            if type(inst).__name__ != "InstMemset":
                continue
            group = []
            ln = getattr(inst.debug, "lineno", None)
            for nxt in block_insts[idx + 1:]:
                if (type(nxt).__name__ == "InstMemset"
                        and str(nxt.engine) == str(inst.engine)
                        and getattr(nxt.debug, "lineno", None) == ln):
                    group.append(nxt)
                    if len(group) >= 3:
                        break
                else:
                    break
            succ_memsets[inst.name] = group
        for inst in block_insts:
            si = inst.sync_info
            if si is None:
                continue
            tname = type(inst).__name__
            q = str(inst.engine)
            clk = dict(queue_clock.get(q, {}))
            def droppable(w):
                return (
                    w.wait_mode == "sem-ge-imm"
                    and w.wait_reg is None
                    and w.wait_value > 0
                    and "barrier" not in w.ant_name
                )

            simple = all(
                w.wait_mode == "sem-ge-imm" and w.wait_reg is None
                for w in si.on_wait
            )
            kept = list(si.on_wait)
            if simple and not any(p in tname for p in _PSEUDO):
                # (a) drop waits the queue has already observed
                kept = [
                    w for w in kept
                    if not (droppable(w) and clk.get(w.ant_name, -1) >= w.wait_value)
                ]
                # (b) drop waits implied by other kept waits
                changed = True
                while changed and len(kept) > 1:
                    changed = False
                    for w in list(kept):
                        if not droppable(w):
                            continue
                        others = [o for o in kept if o is not w]
                        for o in others:
                            oc = clock_at(o.ant_name, o.wait_value)
                            if oc and oc.get(w.ant_name, -1) >= w.wait_value:
                                kept.remove(w)
                                changed = True
                                break
                        if changed:
                            break
                if len(kept) > 1 and all(droppable(w) for w in kept):
                    # spread excess waits onto following same-line memsets
                    # (they write only padding bytes; the protected consumer
                    # follows all of them in queue order)
                    for nxt in succ_memsets.get(inst.name, []):
                        if len(kept) <= 1:
                            break
                        w = kept.pop()
                        nsi = nxt.sync_info
                        nxt.sync_info = bass_rust.SyncInfo(
                            on_wait=list(nsi.on_wait) + [w],
                            on_update=list(nsi.on_update),
                        )
                if len(kept) != len(si.on_wait):
                    inst.sync_info = bass_rust.SyncInfo(
                        on_wait=kept, on_update=list(si.on_update)
                    )
                if len(kept) > 1:
                    n_multi += 1
            # propagate queue clock (history itself was prebuilt in pass A)
            for w in si.on_wait:
                if w.wait_mode == "sem-ge-imm" and w.wait_reg is None:
                    wc = clock_at(w.ant_name, w.wait_value)
                    if wc:
                        join(clk, wc)
                    if clk.get(w.ant_name, -1) < w.wait_value:
                        clk[w.ant_name] = w.wait_value
            queue_clock[q] = clk
    return n_multi


def _build(Tloc, wdt_name):
    """Build the Bass program for Tloc timesteps. wdt_name: 'f32' or 'bf16'."""
    wdt = F32 if wdt_name == "f32" else BF16
    nc = bass.Bass()

    x_d = nc.declare_dram_parameter("x", [I + 1, Tloc * BC], wdt, isOutput=False)
    w1x_d = nc.declare_dram_parameter("w1x", [I + 1, BB], wdt, isOutput=False)
    w1h_d = nc.declare_dram_parameter("w1h", [U, BB], wdt, isOutput=False)
    wf_d = nc.declare_dram_parameter("wf", [BB, NH], wdt, isOutput=False)
    plate_d = nc.declare_dram_parameter("plate", [128, 96], F32, isOutput=False)
    y_d = nc.declare_dram_parameter("y", [BC, Tloc, U], F32, isOutput=True)

    ngroups = Tloc // GS
    assert Tloc % GS == 0

    def sp_dma(out_ap, in_ap, dep):
        """dma_start preceded by an SP no-op that carries the data/WAR wait,
        so the DMA itself keeps only its ring flow-control wait (HW allows a
        single wait per instruction)."""
        if dep is not None:
            nop = nc.sync.nop()
            add_dep_helper(nop.ins, dep.ins, sync=True, reason="dma pre-wait")
            dma = nc.sync.dma_start(out_ap, in_ap)
            add_dep_helper(dma.ins, nop.ins, sync=False, reason="order after pre-wait")
        else:
            dma = nc.sync.dma_start(out_ap, in_ap)
        return dma

    with TileContext(nc) as tc:
        with (
            tc.tile_pool(name="const", bufs=1) as cpool,
            tc.tile_pool(name="xin", bufs=3) as xpool,
            tc.tile_pool(name="zt", bufs=3) as zpool,
            tc.tile_pool(name="ff", bufs=3) as ffpool,
            tc.tile_pool(name="blend", bufs=3) as bpool,
            tc.tile_pool(name="hchunk", bufs=3) as hpool,
            tc.tile_pool(name="ysb", bufs=3) as ypool,
            tc.tile_pool(name="xpre", bufs=2, space="PSUM") as xprepool,
            tc.tile_pool(name="p2", bufs=2, space="PSUM") as p2pool,
            tc.tile_pool(name="yps", bufs=2, space="PSUM") as ypspool,
        ):
            # ---- constants ----
            w1x_t = []
            w1h_t = []
            for m in range(2):
                w = cpool.tile([I + 1, 128], wdt, name=f"w1x{m}")
                nc.sync.dma_start(w[:], w1x_d[:, ts(m, 128)])
                w1x_t.append(w)
                w2 = cpool.tile([U, 128], wdt, name=f"w1h{m}")
                nc.sync.dma_start(w2[:], w1h_d[:, ts(m, 128)])
                w1h_t.append(w2)
            wf_t = [[None] * 3 for _ in range(2)]
            for k in range(2):
                for m in range(3):
                    w = cpool.tile([128, 128], wdt, name=f"wf{k}{m}")
                    nc.sync.dma_start(w[:], wf_d[ts(k, 128), ts(m, 128)])
                    wf_t[k][m] = w
            plate = cpool.tile([128, 96], F32, name="plate")
            nc.sync.dma_start(plate[:], plate_d[:])
            ident = cpool.tile([128, 128], wdt, name="ident")
            make_identity(nc, ident)

            # ---- preamble: settle every const on every consuming engine ----
            # This walrus build allows only ONE sync-wait per executable
            # instruction, so each engine must observe the const-DMA (and
            # cross-engine setup) semaphores via throwaway ops before the
            # loop; in-loop instructions then carry at most one new wait.
            zbias = cpool.tile([128, 1], F32, name="zbias")
            nc.vector.memset(zbias[:], 0.0)
            scratch = cpool.tile([128, 1], F32, name="scratch")
            nc.vector.tensor_copy(scratch[:], plate[:, 0:1])      # DVE sees plate DMA
            scr_a = cpool.tile([128, 1], F32, name="scr_a")
            nc.scalar.copy(scr_a[:], zbias[:])                     # ACT sees DVE
            scr_y = cpool.tile([1, 1], F32, name="scr_y")
            for i, w in enumerate(w1x_t + w1h_t + [t for row in wf_t for t in row]):
                dps = ypspool.tile([128, 1], F32, name="yp")
                nc.tensor.matmul(dps[:], w[:], w[:, 0:1],
                                 start=True, stop=True)            # PE sees each w DMA
            dtr = ypspool.tile([128, 128], wdt, name="yp")
            nc.tensor.transpose(dtr[:], ident[:], ident[:])        # PE sees GPSIMD

            # y DRAM view: t = blk*YB + l ;  yb tile partition = l*BC + b
            y_view = y_d.rearrange(
                "b (blk l) u -> blk l b u", blk=Tloc // YB, l=YB
            )

            hprev = None  # AP of previous step's hh [U, BC]
            pre_mms = {}  # group -> last x-projection matmul (for x WAR waits)
            last_ydma = None

            for g in range(ngroups):
                x_sb = xpool.tile([I + 1, GS * BC], wdt, name="x_sb")
                sp_dma(x_sb[:], x_d[:, ts(g, GS * BC)], pre_mms.get(g - 3))

                # x-projection for this group straight into PSUM (2 banks).
                xp = xprepool.tile([128, 2, 512], F32, name="xp")
                for m in range(2):
                    mm = nc.tensor.matmul(
                        xp[:, m, :], w1x_t[m][:], x_sb[:],
                        start=True, stop=False, skip_group_check=True,
                    )
                pre_mms[g] = mm

                for lt in range(GS):
                    t = g * GS + lt
                    # stage 1: accumulate h-part onto x-projection
                    if t > 0:
                        for m in range(2):
                            nc.tensor.matmul(
                                xp[:, m, ts(lt, BC)], w1h_t[m][:], hprev,
                                start=False, stop=True, skip_group_check=True,
                            )
                    z = zpool.tile([128, 2, BC], wdt, name="z")
                    nc.scalar.activation(
                        z[:], xp[:, :, ts(lt, BC)],
                        mybir.ActivationFunctionType.Tanh,
                        bias=zbias[:], scale=0.666,
                    )
                    # stage 2: three heads, K=256 in two chunks
                    p2 = p2pool.tile([128, 3, BC], F32, name="p2")
                    for m in range(3):
                        for k in range(2):
                            nc.tensor.matmul(
                                p2[:, m, :], wf_t[k][m][:], z[:, k, :],
                                start=(k == 0), stop=(k == 1),
                            )
                    tmp2 = bpool.tile([128, 3, BC], F32, name="tmp2")
                    nc.vector.tensor_add(tmp2[:], p2[:], plate[:])
                    ff = ffpool.tile([128, 3, BC], F32, name="ff")
                    nc.scalar.activation(
                        ff[:], tmp2[:], mybir.ActivationFunctionType.Tanh,
                        bias=zbias[:],
                    )
                    # blend: hh = 2*ff1 + (1+tau)*(ff2-ff1)
                    if lt % YB == 0:
                        hc = hpool.tile([128, YB * BC], wdt, name="hc")
                    d = bpool.tile([128, BC], F32, name="d")
                    nc.vector.tensor_sub(d[:], ff[:, 1, :], ff[:, 0, :])
                    u = bpool.tile([128, BC], F32, name="u")
                    nc.vector.scalar_tensor_tensor(
                        u[:], ff[:, 2, :], 1.0, d[:],
                        mybir.AluOpType.add, mybir.AluOpType.mult,
                    )
                    hslot = hc[:, ts(lt % YB, BC)]
                    nc.vector.scalar_tensor_tensor(
                        hslot, ff[:, 0, :], 2.0, u[:],
                        mybir.AluOpType.mult, mybir.AluOpType.add,
                    )
                    hprev = hslot

                    if lt % YB == YB - 1:
                        # last column is padding no DMA reads: the absorber
                        # memset (write) carries the slot's WAR wait on the
                        # old y-DMA, so the y-copy keeps a single wait.
                        yb = ypool.tile([128, 129], F32, name="yb")
                        nc.vector.memset(yb[0:1, 128:129], 0.0)
                        yp = ypspool.tile([128, 128], wdt, name="yp")
                        nc.tensor.transpose(yp[:], hc[:], ident[:])
                        yc = nc.vector.tensor_scalar_mul(yb[:, 0:128], yp[:], 0.5)
                        last_ydma = sp_dma(y_view[t // YB], yb[:, 0:128], yc)

            # Tail collector: one tiny DMA per HWDGE queue (FIFO per queue),
            # each observed by a DVE memset, so the auto-generated kernel
            # drain's many DMA waits become transitively redundant and are
            # stripped by _minimize_waits (the drain exceeds the wait limit
            # otherwise).
            fins = []
            for k in range(8):
                sk = cpool.tile([1, 1], F32, name=f"fin{k}")
                # read back the last y element: forces these after the final
                # y store, so they are last in every HWDGE queue's FIFO
                sp_dma(sk[:], y_d[BC - 1:BC, Tloc - 1, U - 1:U], last_ydma)
                fins.append(sk)
            for sk in fins:
                nc.vector.memset(sk[:], 0.0)

    n_multi = _minimize_waits(nc)
    if n_multi:
        import warnings
        warnings.warn(f"_minimize_waits: {n_multi} instructions still have >1 wait")
    return nc


def _prep_host(x, W_bb, b_bb, W_ff1, b_ff1, W_ff2, b_ff2, W_ta, b_ta, W_tb, b_tb,
               wdt_name):
    """Host-side preprocessing. Returns per-core input maps."""
    np_wdt = np.float32 if wdt_name == "f32" else ml_dtypes.bfloat16

    W_bb = np.asarray(W_bb, np.float32)
    w1x = np.concatenate([W_bb[:I], np.asarray(b_bb, np.float32)[None, :]], 0)  # [65, 256]
    w1h = 0.5 * W_bb[I:]                                                        # [128, 256]
    s = np.float32(1.7159)
    wf = np.concatenate(
        [s * np.asarray(W_ff1, np.float32),
         s * np.asarray(W_ff2, np.float32),
         s * 0.5 * (np.asarray(W_ta, np.float32) + np.asarray(W_tb, np.float32))],
        axis=1,
    )                                                                           # [256, 384]
    bf = np.concatenate(
        [np.asarray(b_ff1, np.float32),
         np.asarray(b_ff2, np.float32),
         0.5 * (np.asarray(b_ta, np.float32) + np.asarray(b_tb, np.float32))]
    )                                                                           # [384]
    plate = np.empty((128, 96), np.float32)
    for m in range(3):
        plate[:, m * BC:(m + 1) * BC] = bf[m * 128:(m + 1) * 128][:, None]

    w1x = np.ascontiguousarray(w1x.astype(np_wdt))
    w1h = np.ascontiguousarray(w1h.astype(np_wdt))
    wf = np.ascontiguousarray(wf.astype(np_wdt))

    x = np.asarray(x, np.float32)
    in_maps = []
    for c in range(NCORES):
        xc = x[c * BC:(c + 1) * BC]                    # [32, T, 64]
        xf = np.empty((I + 1, xc.shape[1], BC), np.float32)
        xf[:I] = xc.transpose(2, 1, 0)                 # [64, T, 32]
        xf[I] = 1.0
        xf = xf.reshape(I + 1, xc.shape[1] * BC).astype(np_wdt)
        in_maps.append({
            "x": np.ascontiguousarray(xf),
            "w1x": w1x, "w1h": w1h, "wf": wf, "plate": plate,
        })
    return in_maps


last_run_info = {}


def kernel(x, W_bb, b_bb, W_ff1, b_ff1, W_ff2, b_ff2, W_ta, b_ta, W_tb, b_tb):
    import time
    from concourse.bass_utils import run_bass_kernel_spmd

    wdt_name = os.environ.get("CFC_WDT", "bf16")
    trace = os.environ.get("CFC_TRACE", "0") == "1"
    Tloc = x.shape[1]

    key = (Tloc, wdt_name)
    if key not in _CACHE:
        t0 = time.time()
        _CACHE[key] = _build(Tloc, wdt_name)
        last_run_info["build_s"] = time.time() - t0
    nc = _CACHE[key]

    in_maps = _prep_host(x, W_bb, b_bb, W_ff1, b_ff1, W_ff2, b_ff2,
                         W_ta, b_ta, W_tb, b_tb, wdt_name)
    t0 = time.time()
    res = run_bass_kernel_spmd(nc, in_maps, list(range(NCORES)), trace=trace)
    last_run_info["run_s"] = time.time() - t0
    last_run_info["exec_time_ns"] = res.exec_time_ns
    outs = res.results

    readout = np.empty((x.shape[0], Tloc, U), np.float32)
    for c in range(NCORES):
        readout[c * BC:(c + 1) * BC] = outs[c]["y"]
    h_last = np.ascontiguousarray(readout[:, -1, :])
    return readout, h_last
